# revision 1
# baseline (speedup 1.0000x reference)
"""Masked-linear kernel for trn2: out = x @ (mask.T * w) + b.

Full shapes: x (8192, 3072) f32, w (3072, 1536) f32, b (1536,) f32,
mask (1536, 3072) f32 -> out (8192, 1536) f32.

Strategy: 8 NeuronCores as a 4 (batch) x 2 (units) grid. Each core gets
xT (3072, 2048) bf16, w / mask.T shards (3072, 768) bf16, b shard, and
computes outT (768, 2048) f32 = (w*maskT).T @ x_shard.T + b on device:
the mask multiply runs on VectorE, the matmul on TensorE (bf16 with f32
PSUM accumulation, K split into segments accumulated in SBUF f32).
Host only slices / transposes / casts (layout) and reassembles.
"""

import os
import sys

import numpy as np
import ml_dtypes

for _p in ("/opt/trn_rl_repo",):
    if os.path.isdir(_p) and _p not in sys.path:
        sys.path.append(_p)

import concourse.bass as bass  # noqa: E402
import concourse.mybir as mybir  # noqa: E402
import concourse.tile as tile  # noqa: E402
from concourse import bacc  # noqa: E402
from concourse.bass_utils import run_bass_kernel_spmd  # noqa: E402

BF16 = ml_dtypes.bfloat16

BATCH, IN_DIM, UNITS = 8192, 3072, 1536
BW, UW = 4, 2  # batch ways x unit ways = 8 cores
BC = BATCH // BW  # 2048 batch rows per core
UC = UNITS // UW  # 768 units per core
P = 128
K_CHUNKS = IN_DIM // P  # 24
KPS = 4  # K chunks per PSUM accumulation segment
SEGS = K_CHUNKS // KPS  # 6
BT = 512  # matmul moving free dim (one PSUM bank of f32)
NB = BC // BT  # 4
NU = UC // P  # 6
N_CORES = 8

_NC_CACHE = None


def _build_module():
    nc = bacc.Bacc("TRN2", target_bir_lowering=False, debug=False)

    xT = nc.dram_tensor("xT", (IN_DIM, BC), mybir.dt.bfloat16, kind="ExternalInput")
    wp = nc.dram_tensor("wp", (IN_DIM, UC), mybir.dt.bfloat16, kind="ExternalInput")
    mp = nc.dram_tensor("mp", (IN_DIM, UC), mybir.dt.bfloat16, kind="ExternalInput")
    bp = nc.dram_tensor("bp", (P, NU), mybir.dt.float32, kind="ExternalInput")
    outT = nc.dram_tensor("outT", (UC, BC), mybir.dt.float32, kind="ExternalOutput")

    xT3 = xT.ap().rearrange("(ko p) b -> ko p b", p=P)  # [24, 128, 2048]
    wp3 = wp.ap().rearrange("(ko p) u -> ko p u", p=P)  # [24, 128, 768]
    mp3 = mp.ap().rearrange("(ko p) u -> ko p u", p=P)
    oT3 = outT.ap().rearrange("(uo p) b -> uo p b", p=P)  # [6, 128, 2048]

    with tile.TileContext(nc) as tc:
        with (
            tc.tile_pool(name="xpool", bufs=2 * KPS) as xpool,
            tc.tile_pool(name="wpool", bufs=3) as wpool,
            tc.tile_pool(name="mwpool", bufs=2 * KPS) as mwpool,
            tc.tile_pool(name="opool", bufs=1) as opool,
            tc.tile_pool(name="cpool", bufs=1) as cpool,
            tc.tile_pool(name="pspool", bufs=8, space="PSUM") as pspool,
        ):
            btile = cpool.tile([P, NU], mybir.dt.float32, name="btile")
            nc.sync.dma_start(btile[:], bp.ap())

            # persistent f32 output accumulators, one per u-chunk (6 MB)
            out_sb = [
                opool.tile([P, BC], mybir.dt.float32, name=f"osb{u}", tag=f"osb{u}")
                for u in range(NU)
            ]

            for s in range(SEGS):
                xs, mws = [], []
                for kk in range(KPS):
                    k = s * KPS + kk
                    xt = xpool.tile([P, BC], mybir.dt.bfloat16, name=f"xt{k}", tag="xt")
                    nc.sync.dma_start(xt[:], xT3[k])
                    wt = wpool.tile([P, UC], mybir.dt.bfloat16, name=f"wt{k}", tag="wt")
                    nc.sync.dma_start(wt[:], wp3[k])
                    mt = wpool.tile([P, UC], mybir.dt.bfloat16, name=f"mt{k}", tag="mt")
                    nc.sync.dma_start(mt[:], mp3[k])
                    mw = mwpool.tile(
                        [P, UC], mybir.dt.bfloat16, name=f"mw{k}", tag="mw"
                    )
                    nc.vector.tensor_mul(mw[:], wt[:], mt[:])
                    xs.append(xt)
                    mws.append(mw)

                for u in range(NU):
                    ptiles = [
                        pspool.tile(
                            [P, BT], mybir.dt.float32, name=f"ps{s}_{u}_{b}", tag="ps"
                        )
                        for b in range(NB)
                    ]
                    for kk in range(KPS):
                        lhsT = mws[kk][:, u * P : (u + 1) * P]
                        for b in range(NB):
                            nc.tensor.matmul(
                                ptiles[b][:],
                                lhsT,
                                xs[kk][:, b * BT : (b + 1) * BT],
                                start=(kk == 0),
                                stop=(kk == KPS - 1),
                            )
                    for b in range(NB):
                        osl = out_sb[u][:, b * BT : (b + 1) * BT]
                        if s == 0:
                            nc.vector.tensor_add(
                                osl,
                                ptiles[b][:],
                                btile[:, u : u + 1].to_broadcast((P, BT)),
                            )
                        else:
                            nc.vector.tensor_add(osl, osl, ptiles[b][:])
                        if s == SEGS - 1:
                            nc.sync.dma_start(oT3[u][:, b * BT : (b + 1) * BT], osl)

    nc.compile()
    return nc


def get_module():
    global _NC_CACHE
    if _NC_CACHE is None:
        _NC_CACHE = _build_module()
    return _NC_CACHE


def make_in_maps(x, w, b, mask):
    x16 = x.astype(BF16)
    w16 = w.astype(BF16)
    m16T = np.ascontiguousarray(mask.astype(BF16).T)  # (3072, 1536)
    in_maps = []
    for c in range(N_CORES):
        bc, uc = divmod(c, UW)
        in_maps.append(
            {
                "xT": np.ascontiguousarray(x16[bc * BC : (bc + 1) * BC].T),
                "wp": np.ascontiguousarray(w16[:, uc * UC : (uc + 1) * UC]),
                "mp": np.ascontiguousarray(m16T[:, uc * UC : (uc + 1) * UC]),
                "bp": np.ascontiguousarray(
                    b[uc * UC : (uc + 1) * UC].astype(np.float32).reshape(NU, P).T
                ),
            }
        )
    return in_maps


def assemble(results):
    out = np.empty((BATCH, UNITS), dtype=np.float32)
    for c in range(N_CORES):
        bc, uc = divmod(c, UW)
        out[bc * BC : (bc + 1) * BC, uc * UC : (uc + 1) * UC] = results[c]["outT"].T
    return out


def kernel(x, w, b, mask, _trace=False, _trace_kwargs=None):
    x = np.asarray(x, dtype=np.float32)
    w = np.asarray(w, dtype=np.float32)
    b = np.asarray(b, dtype=np.float32)
    mask = np.asarray(mask, dtype=np.float32)
    nc = get_module()
    in_maps = make_in_maps(x, w, b, mask)
    res = run_bass_kernel_spmd(
        nc,
        in_maps,
        core_ids=list(range(N_CORES)),
        trace=_trace,
        **(_trace_kwargs or {}),
    )
    out = assemble(res.results)
    if _trace:
        return out, res
    return out



# revision 2
# speedup vs baseline: 1.2448x; 1.2448x over previous
"""Masked-linear kernel for trn2: out = x @ (mask.T * w) + b.

Full shapes: x (8192, 3072) f32, w (3072, 1536) f32, b (1536,) f32,
mask (1536, 3072) f32 -> out (8192, 1536) f32.

Strategy: 8-way batch-parallel. Each core gets xT (3072, 1024) bf16 and
the full (zero-block-skipped) masked weights, and computes
outT (1536, 1024) bf16 = (w*maskT).T @ x_shard.T + b.

The mask is block-structured (3 unit blocks x 6 input blocks of 512)
with 4 blocks having keep-prob 0 -> those weights are exactly zero and
are skipped entirely (not loaded, not multiplied, not matmul'd):
  units [0,512)     (A): input k-chunks 0-7, 12-19   (16 of 24)
  units [512,1024)  (B): all 24 k-chunks
  units [1024,1536) (C): input k-chunks 4-11, 16-23  (16 of 24)
This cuts the matmul stream from 576 to 448 LDWEIGHTS+MATMUL pairs.

Device pipeline: w/mask arrive in packed layouts that already exclude
zero blocks; VectorE forms mw = w*m; TensorE runs full-K PSUM
accumulation chains (one chain per (u-chunk, batch-half) = one PSUM
bank, 16 or 24 chained matmuls), in groups of 8 chains so all 8 PSUM
banks cycle; VectorE drains each finished bank once (bias add + cast
to bf16); outputs DMA out as they are produced. Host only slices /
transposes / casts (layout) and reassembles.
"""

import os
import sys

import numpy as np
import ml_dtypes

for _p in ("/opt/trn_rl_repo",):
    if os.path.isdir(_p) and _p not in sys.path:
        sys.path.append(_p)

import concourse.bass as bass  # noqa: E402
import concourse.mybir as mybir  # noqa: E402
import concourse.tile as tile  # noqa: E402
from concourse import bacc  # noqa: E402
from concourse.bass_utils import run_bass_kernel_spmd  # noqa: E402

BF16 = ml_dtypes.bfloat16

BATCH, IN_DIM, UNITS = 8192, 3072, 1536
N_CORES = 8
BC = BATCH // N_CORES  # 1024 batch rows per core
P = 128
K_CHUNKS = IN_DIM // P  # 24
BT = 512  # matmul moving free dim (one PSUM bank of f32)
NB = BC // BT  # 2 batch halves per core

# k-chunk lists per unit block (zero blocks skipped)
KA = list(range(0, 8)) + list(range(12, 20))  # units [0,512)
KB = list(range(24))  # units [512,1024)
KC = list(range(4, 12)) + list(range(16, 24))  # units [1024,1536)
# (k-list, unit column range, n quarters) per stage; each stage covers
# 4 u-chunks of 128 units
STAGES = [
    (KA, 0, 4),  # wA: (128, 16*512)
    (KB, 512, 4),  # wB: (128, 24*512)
    (KC, 1024, 4),  # wC: (128, 16*512)
]

_NC_CACHE = None


def _build_module():
    nc = bacc.Bacc("TRN2", target_bir_lowering=False, debug=False)

    xT = nc.dram_tensor("xT", (IN_DIM, BC), mybir.dt.bfloat16, kind="ExternalInput")
    wA = nc.dram_tensor("wA", (P, len(KA) * 512), mybir.dt.bfloat16, kind="ExternalInput")
    mA = nc.dram_tensor("mA", (P, len(KA) * 512), mybir.dt.bfloat16, kind="ExternalInput")
    wB = nc.dram_tensor("wB", (P, len(KB) * 512), mybir.dt.bfloat16, kind="ExternalInput")
    mB = nc.dram_tensor("mB", (P, len(KB) * 512), mybir.dt.bfloat16, kind="ExternalInput")
    wC = nc.dram_tensor("wC", (P, len(KC) * 512), mybir.dt.bfloat16, kind="ExternalInput")
    mC = nc.dram_tensor("mC", (P, len(KC) * 512), mybir.dt.bfloat16, kind="ExternalInput")
    bp = nc.dram_tensor("bp", (P, 12), mybir.dt.float32, kind="ExternalInput")
    outT = nc.dram_tensor("outT", (UNITS, BC), mybir.dt.bfloat16, kind="ExternalOutput")

    xT3 = xT.ap().rearrange("(ko p) b -> ko p b", p=P)  # [24, 128, 1024]
    oT3 = outT.ap().rearrange("(uo p) b -> uo p b", p=P)  # [12, 128, 1024]
    wdr = [wA.ap(), wB.ap(), wC.ap()]
    mdr = [mA.ap(), mB.ap(), mC.ap()]

    with tile.TileContext(nc) as tc:
        with (
            tc.tile_pool(name="xpool", bufs=1) as xpool,
            tc.tile_pool(name="rawpool", bufs=2) as rawpool,
            tc.tile_pool(name="mwpool", bufs=1) as mwpool,
            tc.tile_pool(name="opool", bufs=4) as opool,
            tc.tile_pool(name="cpool", bufs=1) as cpool,
            tc.tile_pool(name="pspool", bufs=8, space="PSUM") as pspool,
        ):
            btile = cpool.tile([P, 12], mybir.dt.float32, name="btile")
            nc.sync.dma_start(btile[:], bp.ap())

            # persistent x tiles (one per k-chunk, 2KB/partition each)
            xt = [
                xpool.tile([P, BC], mybir.dt.bfloat16, name=f"xt{k}", tag=f"xt{k}")
                for k in range(K_CHUNKS)
            ]
            # persistent masked-weight quarter tiles per stage
            mwq = []  # mwq[stage][q] -> [P, qcols]
            for s, (klist, _, nq) in enumerate(STAGES):
                cols = len(klist) * 512
                qcols = cols // nq
                mwq.append(
                    [
                        mwpool.tile(
                            [P, qcols],
                            mybir.dt.bfloat16,
                            name=f"mw{s}_{q}",
                            tag=f"mw{s}_{q}",
                        )
                        for q in range(nq)
                    ]
                )

            def load_stage_quarter(s, q):
                klist, _, nq = STAGES[s]
                cols = len(klist) * 512
                qcols = cols // nq
                wt = rawpool.tile([P, qcols], mybir.dt.bfloat16, name=f"w{s}_{q}", tag="wraw")
                nc.sync.dma_start(wt[:], wdr[s][:, q * qcols : (q + 1) * qcols])
                mt = rawpool.tile([P, qcols], mybir.dt.bfloat16, name=f"m{s}_{q}", tag="mraw")
                nc.sync.dma_start(mt[:], mdr[s][:, q * qcols : (q + 1) * qcols])
                nc.vector.tensor_mul(mwq[s][q][:], wt[:], mt[:])

            def load_x(k):
                nc.sync.dma_start(xt[k][:], xT3[k])

            # --- issue DMAs/muls in consumption order ---
            load_stage_quarter(0, 0)
            for k in KA[:4]:
                load_x(k)
            load_stage_quarter(0, 1)
            for k in KA[4:8]:
                load_x(k)
            load_stage_quarter(0, 2)
            for k in KA[8:12]:
                load_x(k)
            load_stage_quarter(0, 3)
            for k in KA[12:]:
                load_x(k)
            for q in range(4):
                load_stage_quarter(1, q)
            for k in range(K_CHUNKS):
                if k not in KA:
                    load_x(k)
            for q in range(4):
                load_stage_quarter(2, q)

            # --- matmul chain groups: per stage, 4 u-chunks x 2 b = 8 banks ---
            for s, (klist, ucol0, nq) in enumerate(STAGES):
                nk = len(klist)
                kper = nk // nq  # k-chunks per quarter
                ptiles = [
                    [
                        pspool.tile(
                            [P, BT], mybir.dt.float32, name=f"ps{s}_{u}_{b}", tag="ps"
                        )
                        for b in range(NB)
                    ]
                    for u in range(4)
                ]
                for ko in range(nk):
                    k = klist[ko]
                    q, r = divmod(ko, kper)
                    for u in range(4):
                        lhsT = mwq[s][q][:, r * 512 + u * P : r * 512 + (u + 1) * P]
                        for b in range(NB):
                            nc.tensor.matmul(
                                ptiles[u][b][:],
                                lhsT,
                                xt[k][:, b * BT : (b + 1) * BT],
                                start=(ko == 0),
                                stop=(ko == nk - 1),
                            )
                # drain: bias add + cast to bf16, then DMA out
                for u in range(4):
                    ug = s * 4 + u  # global u-chunk index
                    for b in range(NB):
                        osb = opool.tile(
                            [P, BT], mybir.dt.bfloat16, name=f"o{s}_{u}_{b}", tag="osb"
                        )
                        nc.vector.tensor_add(
                            osb[:],
                            ptiles[u][b][:],
                            btile[:, ug : ug + 1].to_broadcast((P, BT)),
                        )
                        nc.sync.dma_start(
                            oT3[ug][:, b * BT : (b + 1) * BT], osb[:]
                        )

    nc.compile()
    return nc


def get_module():
    global _NC_CACHE
    if _NC_CACHE is None:
        _NC_CACHE = _build_module()
    return _NC_CACHE


def _pack_wm(arr2d, klist, ucol0):
    """arr2d (IN_DIM, UNITS)-like slab -> (128, len(klist)*512) packed
    [p, ko*512 + u] = arr2d[klist[ko]*128 + p, ucol0 + u]."""
    sl = arr2d[:, ucol0 : ucol0 + 512].reshape(K_CHUNKS, P, 512)[klist]
    return np.ascontiguousarray(sl.transpose(1, 0, 2).reshape(P, len(klist) * 512))


def make_in_maps(x, w, b, mask):
    x16 = x.astype(BF16)
    w16 = w.astype(BF16)
    m16T = np.ascontiguousarray(mask.astype(BF16).T)  # (3072, 1536)

    shared = {}
    for name, src in (("w", w16), ("m", m16T)):
        for s, (klist, ucol0, _) in enumerate(STAGES):
            shared[name + "ABC"[s]] = _pack_wm(src, klist, ucol0)
    shared["bp"] = np.ascontiguousarray(
        b.astype(np.float32).reshape(12, P).T
    )

    in_maps = []
    for c in range(N_CORES):
        m = dict(shared)
        m["xT"] = np.ascontiguousarray(x16[c * BC : (c + 1) * BC].T)
        in_maps.append(m)
    return in_maps


def assemble(results):
    out = np.empty((BATCH, UNITS), dtype=np.float32)
    for c in range(N_CORES):
        out[c * BC : (c + 1) * BC, :] = results[c]["outT"].T
    return out


def kernel(x, w, b, mask, _trace=False, _trace_kwargs=None):
    x = np.asarray(x, dtype=np.float32)
    w = np.asarray(w, dtype=np.float32)
    b = np.asarray(b, dtype=np.float32)
    mask = np.asarray(mask, dtype=np.float32)
    nc = get_module()
    in_maps = make_in_maps(x, w, b, mask)
    res = run_bass_kernel_spmd(
        nc,
        in_maps,
        core_ids=list(range(N_CORES)),
        trace=_trace,
        **(_trace_kwargs or {}),
    )
    out = assemble(res.results)
    if _trace:
        return out, res
    return out


# revision 3
# speedup vs baseline: 1.3534x; 1.0872x over previous
"""Masked-linear kernel for trn2: out = x @ (mask.T * w) + b.

Full shapes: x (8192, 3072) f32, w (3072, 1536) f32, b (1536,) f32,
mask (1536, 3072) f32 -> out (8192, 1536) f32.

Strategy: 8-way batch-parallel. Each core gets xT (3072, 1024) bf16 and
the full (zero-block-skipped) masked weights, and computes
outT (1536, 1024) bf16 = (w*maskT).T @ x_shard.T + b.

The mask is block-structured (3 unit blocks x 6 input blocks of 512)
with 4 blocks having keep-prob 0 -> those weights are exactly zero and
are skipped entirely (not loaded, not multiplied, not matmul'd):
  units [0,512)     (A): input k-chunks 0-7, 12-19   (16 of 24)
  units [512,1024)  (B): all 24 k-chunks
  units [1024,1536) (C): input k-chunks 4-11, 16-23  (16 of 24)
This cuts the matmul stream from 576 to 448 LDWEIGHTS+MATMUL pairs.

Device pipeline: w (SP DMA ring) and mask (ACT DMA ring) stream in as
sub-chunks; VectorE forms mw = w*m per chunk; TensorE runs full-K PSUM
accumulation chains (one chain per (u-chunk, batch-half) = one PSUM
bank, 16 or 24 chained matmuls) in groups of 4 chains so 4 banks
compute while the previous 4 drain; drains (bias add + cast to bf16)
alternate between VectorE and ScalarE; outputs DMA out as produced.
Host only slices / transposes / casts (layout) and reassembles.
"""

import os
import sys

import numpy as np
import ml_dtypes

for _p in ("/opt/trn_rl_repo",):
    if os.path.isdir(_p) and _p not in sys.path:
        sys.path.append(_p)

import concourse.bass as bass  # noqa: E402
import concourse.mybir as mybir  # noqa: E402
import concourse.tile as tile  # noqa: E402
from concourse import bacc  # noqa: E402
from concourse.bass_utils import run_bass_kernel_spmd  # noqa: E402

BF16 = ml_dtypes.bfloat16

BATCH, IN_DIM, UNITS = 8192, 3072, 1536
N_CORES = 8
BC = BATCH // N_CORES  # 1024 batch rows per core
P = 128
K_CHUNKS = IN_DIM // P  # 24
BT = 512  # matmul moving free dim (one PSUM bank of f32)
NB = BC // BT  # 2 batch halves per core

# k-chunk lists per unit block (zero blocks skipped)
KA = list(range(0, 8)) + list(range(12, 20))  # units [0,512)
KB = list(range(24))  # units [512,1024)
KC = list(range(4, 12)) + list(range(16, 24))  # units [1024,1536)
STAGES = [(KA, 0), (KB, 512), (KC, 1024)]
# w/mask DMA+mul sub-chunks per stage, as ko ranges
CHUNKS = [
    [(0, 2), (2, 8), (8, 16)],
    [(0, 8), (8, 16), (16, 24)],
    [(0, 8), (8, 16)],
]

_NC_CACHE = None


def _build_module():
    nc = bacc.Bacc("TRN2", target_bir_lowering=False, debug=False)

    xT = nc.dram_tensor("xT", (IN_DIM, BC), mybir.dt.bfloat16, kind="ExternalInput")
    wd, md = [], []
    for s, (klist, _) in enumerate(STAGES):
        wd.append(
            nc.dram_tensor(
                f"w{s}", (P, len(klist) * 512), mybir.dt.bfloat16, kind="ExternalInput"
            )
        )
        md.append(
            nc.dram_tensor(
                f"m{s}", (P, len(klist) * 512), mybir.dt.bfloat16, kind="ExternalInput"
            )
        )
    bp = nc.dram_tensor("bp", (P, 12), mybir.dt.float32, kind="ExternalInput")
    outT = nc.dram_tensor("outT", (UNITS, BC), mybir.dt.bfloat16, kind="ExternalOutput")

    xT3 = xT.ap().rearrange("(ko p) b -> ko p b", p=P)  # [24, 128, 1024]
    oT3 = outT.ap().rearrange("(uo p) b -> uo p b", p=P)  # [12, 128, 1024]

    with tile.TileContext(nc) as tc:
        with (
            tc.tile_pool(name="xpool", bufs=1) as xpool,
            tc.tile_pool(name="rawpool", bufs=2) as rawpool,
            tc.tile_pool(name="mwpool", bufs=1) as mwpool,
            tc.tile_pool(name="opool", bufs=6) as opool,
            tc.tile_pool(name="cpool", bufs=1) as cpool,
            tc.tile_pool(name="pspool", bufs=8, space="PSUM") as pspool,
        ):
            btile = cpool.tile([P, 12], mybir.dt.float32, name="btile")
            nc.sync.dma_start(btile[:], bp.ap())

            # persistent x tiles (one per k-chunk, 2KB/partition each)
            xt = [
                xpool.tile([P, BC], mybir.dt.bfloat16, name=f"xt{k}", tag=f"xt{k}")
                for k in range(K_CHUNKS)
            ]
            # persistent masked-weight tile per stage
            mw = [
                mwpool.tile(
                    [P, len(klist) * 512],
                    mybir.dt.bfloat16,
                    name=f"mw{s}",
                    tag=f"mw{s}",
                )
                for s, (klist, _) in enumerate(STAGES)
            ]

            def load_wm_chunk(s, c0, c1):
                cols = (c1 - c0) * 512
                wt = rawpool.tile([P, cols], mybir.dt.bfloat16, name=f"w{s}_{c0}", tag="wraw")
                nc.sync.dma_start(wt[:], wd[s][:, c0 * 512 : c1 * 512])
                mt = rawpool.tile([P, cols], mybir.dt.bfloat16, name=f"m{s}_{c0}", tag="mraw")
                nc.scalar.dma_start(mt[:], md[s][:, c0 * 512 : c1 * 512])
                nc.vector.tensor_mul(mw[s][:, c0 * 512 : c1 * 512], wt[:], mt[:])

            def load_x(k):
                nc.sync.dma_start(xt[k][:], xT3[k])

            # --- issue DMAs/muls in consumption order ---
            load_wm_chunk(0, *CHUNKS[0][0])
            for k in KA[:2]:
                load_x(k)
            load_wm_chunk(0, *CHUNKS[0][1])
            for k in KA[2:8]:
                load_x(k)
            load_wm_chunk(0, *CHUNKS[0][2])
            for k in KA[8:16]:
                load_x(k)
            for c0, c1 in CHUNKS[1]:
                load_wm_chunk(1, c0, c1)
            for k in range(K_CHUNKS):
                if k not in KA:
                    load_x(k)
            for c0, c1 in CHUNKS[2]:
                load_wm_chunk(2, c0, c1)

            # --- matmul chain groups: 2 u-chunks x 2 b = 4 banks per group,
            # so 4 banks compute while the previous group's 4 drain ---
            for s, (klist, _) in enumerate(STAGES):
                nk = len(klist)
                for g in range(2):  # u-chunk pairs (0,1) and (2,3) of stage
                    ptiles = [
                        [
                            pspool.tile(
                                [P, BT],
                                mybir.dt.float32,
                                name=f"ps{s}_{g}_{u}_{b}",
                                tag="ps",
                            )
                            for b in range(NB)
                        ]
                        for u in range(2)
                    ]
                    for ko in range(nk):
                        k = klist[ko]
                        for u in range(2):
                            uu = g * 2 + u
                            lhsT = mw[s][:, ko * 512 + uu * P : ko * 512 + (uu + 1) * P]
                            for b in range(NB):
                                nc.tensor.matmul(
                                    ptiles[u][b][:],
                                    lhsT,
                                    xt[k][:, b * BT : (b + 1) * BT],
                                    start=(ko == 0),
                                    stop=(ko == nk - 1),
                                )
                    # drain: bias add + cast to bf16 (VectorE/ScalarE), DMA out
                    for u in range(2):
                        ug = s * 4 + g * 2 + u  # global u-chunk index
                        for b in range(NB):
                            osb = opool.tile(
                                [P, BT],
                                mybir.dt.bfloat16,
                                name=f"o{s}_{g}_{u}_{b}",
                                tag="osb",
                            )
                            bcol = btile[:, ug : ug + 1]
                            if b == 0:
                                nc.vector.tensor_add(
                                    osb[:], ptiles[u][b][:], bcol.to_broadcast((P, BT))
                                )
                            else:
                                nc.scalar.add(osb[:], ptiles[u][b][:], bcol)
                            nc.sync.dma_start(
                                oT3[ug][:, b * BT : (b + 1) * BT], osb[:]
                            )

    nc.compile()
    return nc


def get_module():
    global _NC_CACHE
    if _NC_CACHE is None:
        _NC_CACHE = _build_module()
    return _NC_CACHE


def _pack_wm(arr2d, klist, ucol0):
    """arr2d (IN_DIM, UNITS)-like slab -> (128, len(klist)*512) packed
    [p, ko*512 + u] = arr2d[klist[ko]*128 + p, ucol0 + u]."""
    sl = arr2d[:, ucol0 : ucol0 + 512].reshape(K_CHUNKS, P, 512)[klist]
    return np.ascontiguousarray(sl.transpose(1, 0, 2).reshape(P, len(klist) * 512))


def make_in_maps(x, w, b, mask):
    x16 = x.astype(BF16)
    w16 = w.astype(BF16)
    m16T = np.ascontiguousarray(mask.astype(BF16).T)  # (3072, 1536)

    shared = {}
    for s, (klist, ucol0) in enumerate(STAGES):
        shared[f"w{s}"] = _pack_wm(w16, klist, ucol0)
        shared[f"m{s}"] = _pack_wm(m16T, klist, ucol0)
    shared["bp"] = np.ascontiguousarray(b.astype(np.float32).reshape(12, P).T)

    in_maps = []
    for c in range(N_CORES):
        m = dict(shared)
        m["xT"] = np.ascontiguousarray(x16[c * BC : (c + 1) * BC].T)
        in_maps.append(m)
    return in_maps


def assemble(results):
    out = np.empty((BATCH, UNITS), dtype=np.float32)
    for c in range(N_CORES):
        out[c * BC : (c + 1) * BC, :] = results[c]["outT"].T
    return out


def kernel(x, w, b, mask, _trace=False, _trace_kwargs=None):
    x = np.asarray(x, dtype=np.float32)
    w = np.asarray(w, dtype=np.float32)
    b = np.asarray(b, dtype=np.float32)
    mask = np.asarray(mask, dtype=np.float32)
    nc = get_module()
    in_maps = make_in_maps(x, w, b, mask)
    res = run_bass_kernel_spmd(
        nc,
        in_maps,
        core_ids=list(range(N_CORES)),
        trace=_trace,
        **(_trace_kwargs or {}),
    )
    out = assemble(res.results)
    if _trace:
        return out, res
    return out


# revision 10
# speedup vs baseline: 1.3710x; 1.0130x over previous
"""Masked-linear kernel for trn2: out = x @ (mask.T * w) + b.

Full shapes: x (8192, 3072) f32, w (3072, 1536) f32, b (1536,) f32,
mask (1536, 3072) f32 -> out (8192, 1536) f32.

Strategy: 8-way batch-parallel. Each core gets xT (3072, 1024) bf16 and
the full (zero-block-skipped) masked weights, and computes
outT (1536, 1024) bf16 = (w*maskT).T @ x_shard.T + b.

The mask is block-structured (3 unit blocks x 6 input blocks of 512)
with 4 blocks having keep-prob 0 -> those weights are exactly zero and
are skipped entirely (not loaded, not multiplied, not matmul'd):
  units [0,512)     (A): input k-chunks 0-7, 12-19   (16 of 24)
  units [512,1024)  (B): all 24 k-chunks
  units [1024,1536) (C): input k-chunks 4-11, 16-23  (16 of 24)
This cuts the matmul stream from 576 to 448 LDWEIGHTS+MATMUL pairs.

Device pipeline: a tiny zeros tensor feeds a few warm-up matmuls that
bring the PE out of its cold power state while real data loads; w (SP
DMA ring) and mask (ACT DMA ring) stream in as sub-chunks; VectorE
forms mw = w*m per chunk; TensorE runs full-K PSUM accumulation chains
(one chain per (u-chunk, batch-half) = one PSUM bank, 16 or 24 chained
matmuls). Chains run in groups of 4 = (4 u-chunks x 1 batch-half), so
the early groups only need half of each x chunk (the DMA fabric cannot
feed w+mask+full-x at matmul rate at the head); groups alternate
between two disjoint 4-bank PSUM sets so a group never waits on the
previous group's drains; drains (bias add + cast to bf16) alternate
VectorE/ScalarE into per-u-chunk staging tiles; one output DMA per
u-chunk. Host only slices / transposes / casts (layout) and
reassembles.
"""

import os
import sys

import numpy as np
import ml_dtypes

for _p in ("/opt/trn_rl_repo",):
    if os.path.isdir(_p) and _p not in sys.path:
        sys.path.append(_p)

import concourse.bass as bass  # noqa: E402
import concourse.mybir as mybir  # noqa: E402
import concourse.tile as tile  # noqa: E402
from concourse import bacc  # noqa: E402
from concourse.bass_utils import run_bass_kernel_spmd  # noqa: E402

BF16 = ml_dtypes.bfloat16

BATCH, IN_DIM, UNITS = 8192, 3072, 1536
N_CORES = 8
BC = BATCH // N_CORES  # 1024 batch rows per core
P = 128
K_CHUNKS = IN_DIM // P  # 24
BT = 512  # matmul moving free dim (one PSUM bank of f32)
NB = BC // BT  # 2 batch halves per core
NWARM = 6  # warm-up matmuls at kernel start

# k-chunk lists per unit block (zero blocks skipped)
KA = list(range(0, 8)) + list(range(12, 20))  # units [0,512)
KB = list(range(24))  # units [512,1024)
KC = list(range(4, 12)) + list(range(16, 24))  # units [1024,1536)
STAGES = [(KA, 0), (KB, 512), (KC, 1024)]
# w/mask DMA+mul sub-chunks per stage, as ko ranges
CHUNKS = [
    [(0, 1), (1, 2), (2, 8), (8, 16)],
    [(0, 8), (8, 16), (16, 24)],
    [(0, 8), (8, 16)],
]
# x DMA batches (contiguous k ranges), in stage-A consumption order
XGROUPS = [(0, 1), (1, 2), (2, 4), (4, 8), (12, 16), (16, 20), (8, 12), (20, 24)]

_NC_CACHE = None


def _build_module():
    nc = bacc.Bacc("TRN2", target_bir_lowering=False, debug=False)

    xT = nc.dram_tensor("xT", (IN_DIM, BC), mybir.dt.bfloat16, kind="ExternalInput")
    warm = nc.dram_tensor("warm", (P, BT), mybir.dt.bfloat16, kind="ExternalInput")
    wd, md = [], []
    for s, (klist, _) in enumerate(STAGES):
        wd.append(
            nc.dram_tensor(
                f"w{s}", (P, len(klist) * 512), mybir.dt.bfloat16, kind="ExternalInput"
            )
        )
        md.append(
            nc.dram_tensor(
                f"m{s}", (P, len(klist) * 512), mybir.dt.bfloat16, kind="ExternalInput"
            )
        )
    bp = nc.dram_tensor("bp", (P, 12), mybir.dt.float32, kind="ExternalInput")
    outT = nc.dram_tensor("outT", (UNITS, BC), mybir.dt.bfloat16, kind="ExternalOutput")

    oT3 = outT.ap().rearrange("(uo p) b -> uo p b", p=P)  # [12, 128, 1024]

    with tile.TileContext(nc) as tc:
        with (
            tc.tile_pool(name="xpool", bufs=1) as xpool,
            tc.tile_pool(name="rawpool", bufs=2) as rawpool,
            tc.tile_pool(name="mwpool", bufs=1) as mwpool,
            tc.tile_pool(name="opool", bufs=6) as opool,
            tc.tile_pool(name="cpool", bufs=1) as cpool,
            tc.tile_pool(name="psa", bufs=4, space="PSUM") as psa,
            tc.tile_pool(name="psb", bufs=4, space="PSUM") as psb,
        ):
            # --- warm-up: bring PE to full power state during load ---
            wtile = cpool.tile([P, BT], mybir.dt.bfloat16, name="wtile")
            nc.sync.dma_start(wtile[:], warm.ap())
            wps = psa.tile([P, BT], mybir.dt.float32, name="wps", tag="ps")
            for _ in range(NWARM):
                nc.tensor.matmul(wps[:], wtile[:, 0:P], wtile[:], start=True, stop=True)

            # persistent x storage: (k, b) -> (tile, kp); tiles are
            # [P, nk, 512] per (XGROUP, batch-half)
            xmap = {}
            xtiles = {}
            for gi, (k0, k1) in enumerate(XGROUPS):
                for b in range(NB):
                    t = xpool.tile(
                        [P, k1 - k0, BT],
                        mybir.dt.bfloat16,
                        name=f"xg{gi}_{b}",
                        tag=f"xg{gi}_{b}",
                    )
                    xtiles[(gi, b)] = (t, k0, k1)
                    for k in range(k0, k1):
                        xmap[(k, b)] = (t, k - k0)
            # persistent masked-weight tile per stage
            mw = [
                mwpool.tile(
                    [P, len(klist) * 512],
                    mybir.dt.bfloat16,
                    name=f"mw{s}",
                    tag=f"mw{s}",
                )
                for s, (klist, _) in enumerate(STAGES)
            ]
            btile = cpool.tile([P, 12], mybir.dt.float32, name="btile")

            def load_wm_chunk(s, c0, c1):
                cols = (c1 - c0) * 512
                wt = rawpool.tile([P, cols], mybir.dt.bfloat16, name=f"w{s}_{c0}", tag="wraw")
                nc.sync.dma_start(wt[:], wd[s][:, c0 * 512 : c1 * 512])
                mt = rawpool.tile([P, cols], mybir.dt.bfloat16, name=f"m{s}_{c0}", tag="mraw")
                nc.scalar.dma_start(mt[:], md[s][:, c0 * 512 : c1 * 512])
                nc.vector.tensor_mul(mw[s][:, c0 * 512 : c1 * 512], wt[:], mt[:])

            def load_xg(gi, b):
                t, k0, k1 = xtiles[(gi, b)]
                src = xT.ap()[k0 * P : k1 * P, b * BT : (b + 1) * BT].rearrange(
                    "(kp p) b -> p kp b", p=P
                )
                nc.sync.dma_start(t[:], src)

            # --- issue DMAs/muls in consumption order ---
            load_wm_chunk(0, *CHUNKS[0][0])
            load_xg(0, 0)
            load_wm_chunk(0, *CHUNKS[0][1])
            load_xg(1, 0)
            load_wm_chunk(0, *CHUNKS[0][2])
            load_xg(2, 0)
            load_xg(3, 0)
            load_wm_chunk(0, *CHUNKS[0][3])
            load_xg(4, 0)
            load_xg(5, 0)
            for gi in range(6):  # stage-A b1 tiles
                load_xg(gi, 1)
            nc.scalar.dma_start(btile[:], bp.ap())
            for c0, c1 in CHUNKS[1]:
                load_wm_chunk(1, c0, c1)
            for b in range(NB):  # B-only k chunks
                load_xg(6, b)
                load_xg(7, b)
            for c0, c1 in CHUNKS[2]:
                load_wm_chunk(2, c0, c1)

            # --- matmul chain groups: 4 u-chunks x 1 batch-half = 4 banks,
            # alternating between two disjoint 4-bank PSUM pools ---
            # per-stage staging tiles for output (b0|b1 halves)
            gidx = 0
            for s, (klist, _) in enumerate(STAGES):
                nk = len(klist)
                osbs = [
                    opool.tile(
                        [P, BC], mybir.dt.bfloat16, name=f"o{s}_{u}", tag="osb"
                    )
                    for u in range(4)
                ]
                for b in range(NB):
                    pool = psa if gidx % 2 == 0 else psb
                    ptiles = [
                        pool.tile(
                            [P, BT], mybir.dt.float32, name=f"ps{s}_{b}_{u}", tag="ps"
                        )
                        for u in range(4)
                    ]
                    for ko in range(nk):
                        k = klist[ko]
                        xt_t, kp = xmap[(k, b)]
                        for u in range(4):
                            lhsT = mw[s][:, ko * 512 + u * P : ko * 512 + (u + 1) * P]
                            nc.tensor.matmul(
                                ptiles[u][:],
                                lhsT,
                                xt_t[:, kp, :],
                                start=(ko == 0),
                                stop=(ko == nk - 1),
                            )
                    # drains: bias add + cast to bf16, VectorE/ScalarE split
                    for u in range(4):
                        ug = s * 4 + u  # global u-chunk index
                        bcol = btile[:, ug : ug + 1]
                        dst = osbs[u][:, b * BT : (b + 1) * BT]
                        if u % 2 == 0:
                            nc.vector.tensor_add(
                                dst, ptiles[u][:], bcol.to_broadcast((P, BT))
                            )
                        else:
                            nc.scalar.add(dst, ptiles[u][:], bcol)
                        if b == NB - 1:
                            nc.sync.dma_start(oT3[ug], osbs[u][:])
                    gidx += 1

    nc.compile()
    return nc


def get_module():
    global _NC_CACHE
    if _NC_CACHE is None:
        _NC_CACHE = _build_module()
    return _NC_CACHE


def _pack_wm(arr2d, klist, ucol0):
    """arr2d (IN_DIM, UNITS)-like slab -> (128, len(klist)*512) packed
    [p, ko*512 + u] = arr2d[klist[ko]*128 + p, ucol0 + u]."""
    sl = arr2d[:, ucol0 : ucol0 + 512].reshape(K_CHUNKS, P, 512)[klist]
    return np.ascontiguousarray(sl.transpose(1, 0, 2).reshape(P, len(klist) * 512))


def make_in_maps(x, w, b, mask):
    x16 = x.astype(BF16)
    w16 = w.astype(BF16)
    m16T = np.ascontiguousarray(mask.astype(BF16).T)  # (3072, 1536)

    shared = {"warm": np.zeros((P, BT), dtype=BF16)}
    for s, (klist, ucol0) in enumerate(STAGES):
        shared[f"w{s}"] = _pack_wm(w16, klist, ucol0)
        shared[f"m{s}"] = _pack_wm(m16T, klist, ucol0)
    shared["bp"] = np.ascontiguousarray(b.astype(np.float32).reshape(12, P).T)

    in_maps = []
    for c in range(N_CORES):
        m = dict(shared)
        m["xT"] = np.ascontiguousarray(x16[c * BC : (c + 1) * BC].T)
        in_maps.append(m)
    return in_maps


def assemble(results):
    out = np.empty((BATCH, UNITS), dtype=np.float32)
    for c in range(N_CORES):
        out[c * BC : (c + 1) * BC, :] = results[c]["outT"].T
    return out


def kernel(x, w, b, mask, _trace=False, _trace_kwargs=None):
    x = np.asarray(x, dtype=np.float32)
    w = np.asarray(w, dtype=np.float32)
    b = np.asarray(b, dtype=np.float32)
    mask = np.asarray(mask, dtype=np.float32)
    nc = get_module()
    in_maps = make_in_maps(x, w, b, mask)
    res = run_bass_kernel_spmd(
        nc,
        in_maps,
        core_ids=list(range(N_CORES)),
        trace=_trace,
        **(_trace_kwargs or {}),
    )
    out = assemble(res.results)
    if _trace:
        return out, res
    return out


# revision 14
# speedup vs baseline: 1.3891x; 1.0132x over previous
"""Masked-linear kernel for trn2: out = x @ (mask.T * w) + b.

Full shapes: x (8192, 3072) f32, w (3072, 1536) f32, b (1536,) f32,
mask (1536, 3072) f32 -> out (8192, 1536) f32.

Strategy: 8-way batch-parallel. Each core gets xT (3072, 1024) bf16 and
the full (zero-block-skipped) masked weights, and computes
outT (1536, 1024) bf16 = (w*maskT).T @ x_shard.T + b.

The mask is block-structured (3 unit blocks x 6 input blocks of 512)
with 4 blocks having keep-prob 0 -> those weights are exactly zero and
are skipped entirely (not loaded, not multiplied, not matmul'd):
  units [0,512)     (A): input k-chunks 0-7, 12-19   (16 of 24)
  units [512,1024)  (B): all 24 k-chunks
  units [1024,1536) (C): input k-chunks 4-11, 16-23  (16 of 24)
This cuts the matmul stream from 576 to 448 LDWEIGHTS+MATMUL pairs.

Device pipeline: a tiny zeros tensor feeds a few warm-up matmuls that
bring the PE out of its cold power state while real data loads; w (SP
DMA ring) and mask (ACT DMA ring) stream in as sub-chunks; VectorE
forms mw = w*m per chunk; TensorE runs full-K PSUM accumulation chains
(one chain per (u-chunk, batch-half) = one PSUM bank, 16 or 24 chained
matmuls). Chains run in groups of 4 = (4 u-chunks x 1 batch-half), so
the early groups only need half of each x chunk (the DMA fabric cannot
feed w+mask+full-x at matmul rate at the head); groups alternate
between two disjoint 4-bank PSUM sets so a group never waits on the
previous group's drains; drains (bias add + cast to bf16) alternate
VectorE/ScalarE into per-u-chunk staging tiles; one output DMA per
u-chunk. Host only slices / transposes / casts (layout) and
reassembles.
"""

import os
import sys

import numpy as np
import ml_dtypes

for _p in ("/opt/trn_rl_repo",):
    if os.path.isdir(_p) and _p not in sys.path:
        sys.path.append(_p)

import concourse.bass as bass  # noqa: E402
import concourse.mybir as mybir  # noqa: E402
import concourse.tile as tile  # noqa: E402
from concourse import bacc  # noqa: E402
from concourse.bass_utils import run_bass_kernel_spmd  # noqa: E402

BF16 = ml_dtypes.bfloat16

BATCH, IN_DIM, UNITS = 8192, 3072, 1536
N_CORES = 8
BC = BATCH // N_CORES  # 1024 batch rows per core
P = 128
K_CHUNKS = IN_DIM // P  # 24
BT = 512  # matmul moving free dim (one PSUM bank of f32)
NB = BC // BT  # 2 batch halves per core
NWARM = 6  # warm-up matmuls at kernel start

# k-chunk lists per unit block (zero blocks skipped)
KA = list(range(0, 8)) + list(range(12, 20))  # units [0,512)
KB = list(range(24))  # units [512,1024)
KC = list(range(4, 12)) + list(range(16, 24))  # units [1024,1536)
STAGES = [(KA, 0), (KB, 512), (KC, 1024)]
# w/mask DMA+mul sub-chunks per stage, as ko ranges
CHUNKS = [
    [(0, 1), (1, 2), (2, 8), (8, 16)],
    [(0, 8), (8, 16), (16, 24)],
    [(0, 8), (8, 16)],
]
# x DMA batches (contiguous k ranges), in stage-A consumption order
XGROUPS = [(0, 1), (1, 2), (2, 4), (4, 8), (12, 16), (16, 20), (8, 12), (20, 24)]

_NC_CACHE = None


def _build_module():
    nc = bacc.Bacc("TRN2", target_bir_lowering=False, debug=False)

    xT = nc.dram_tensor("xT", (IN_DIM, BC), mybir.dt.bfloat16, kind="ExternalInput")
    warm = nc.dram_tensor("warm", (P, BT), mybir.dt.bfloat16, kind="ExternalInput")
    wd, md = [], []
    for s, (klist, _) in enumerate(STAGES):
        wd.append(
            nc.dram_tensor(
                f"w{s}", (P, len(klist) * 512), mybir.dt.bfloat16, kind="ExternalInput"
            )
        )
        md.append(
            nc.dram_tensor(
                f"m{s}", (P, len(klist) * 512), mybir.dt.bfloat16, kind="ExternalInput"
            )
        )
    bp = nc.dram_tensor("bp", (P, 12), mybir.dt.float32, kind="ExternalInput")
    outT = nc.dram_tensor("outT", (UNITS, BC), mybir.dt.bfloat16, kind="ExternalOutput")

    oT3 = outT.ap().rearrange("(uo p) b -> uo p b", p=P)  # [12, 128, 1024]

    with tile.TileContext(nc) as tc:
        with (
            tc.tile_pool(name="xpool", bufs=1) as xpool,
            tc.tile_pool(name="rawpool", bufs=2) as rawpool,
            tc.tile_pool(name="mwpool", bufs=1) as mwpool,
            tc.tile_pool(name="opool", bufs=6) as opool,
            tc.tile_pool(name="cpool", bufs=1) as cpool,
            tc.tile_pool(name="psa", bufs=4, space="PSUM") as psa,
            tc.tile_pool(name="psb", bufs=4, space="PSUM") as psb,
        ):
            # --- warm-up: bring PE to full power state during load ---
            wtile = cpool.tile([P, BT], mybir.dt.bfloat16, name="wtile")
            nc.sync.dma_start(wtile[:], warm.ap())
            wps = psa.tile([P, BT], mybir.dt.float32, name="wps", tag="ps")
            for _ in range(NWARM):
                nc.tensor.matmul(wps[:], wtile[:, 0:P], wtile[:], start=True, stop=True)

            # persistent x storage: k -> (tile, kp); tiles are [P, nk, 1024]
            xmap = {}
            xtiles = []
            for gi, (k0, k1) in enumerate(XGROUPS):
                t = xpool.tile(
                    [P, k1 - k0, BC],
                    mybir.dt.bfloat16,
                    name=f"xg{gi}",
                    tag=f"xg{gi}",
                )
                xtiles.append((t, k0, k1))
                for k in range(k0, k1):
                    xmap[k] = (t, k - k0)
            # persistent masked-weight tile per stage
            mw = [
                mwpool.tile(
                    [P, len(klist) * 512],
                    mybir.dt.bfloat16,
                    name=f"mw{s}",
                    tag=f"mw{s}",
                )
                for s, (klist, _) in enumerate(STAGES)
            ]
            btile = cpool.tile([P, 12], mybir.dt.float32, name="btile")

            def load_wm_chunk(s, c0, c1):
                cols = (c1 - c0) * 512
                wt = rawpool.tile([P, cols], mybir.dt.bfloat16, name=f"w{s}_{c0}", tag="wraw")
                nc.sync.dma_start(wt[:], wd[s][:, c0 * 512 : c1 * 512])
                mt = rawpool.tile([P, cols], mybir.dt.bfloat16, name=f"m{s}_{c0}", tag="mraw")
                nc.scalar.dma_start(mt[:], md[s][:, c0 * 512 : c1 * 512])
                nc.vector.tensor_mul(mw[s][:, c0 * 512 : c1 * 512], wt[:], mt[:])

            def load_xg(gi):
                t, k0, k1 = xtiles[gi]
                src = xT.ap()[k0 * P : k1 * P, :].rearrange("(kp p) b -> p kp b", p=P)
                nc.sync.dma_start(t[:], src)

            # --- issue DMAs/muls in consumption order ---
            load_wm_chunk(0, *CHUNKS[0][0])
            load_xg(0)
            load_wm_chunk(0, *CHUNKS[0][1])
            load_xg(1)
            load_wm_chunk(0, *CHUNKS[0][2])
            load_xg(2)
            load_xg(3)
            load_wm_chunk(0, *CHUNKS[0][3])
            load_xg(4)
            load_xg(5)
            nc.scalar.dma_start(btile[:], bp.ap())
            for c0, c1 in CHUNKS[1]:
                load_wm_chunk(1, c0, c1)
            load_xg(6)
            load_xg(7)
            for c0, c1 in CHUNKS[2]:
                load_wm_chunk(2, c0, c1)

            # --- matmul chain groups ---
            # Stage A (first, supply-bound head): one 8-chain group over all
            # 8 PSUM banks -> slowest per-ko demand on the DMA fabric.
            # Stages B/C: 4-chain (4 u-chunks x 1 batch-half) groups
            # alternating between the two 4-bank pools.
            def drain(s, u, b, ptile, osbs, eng_v):
                ug = s * 4 + u
                bcol = btile[:, ug : ug + 1]
                dst = osbs[u][:, b * BT : (b + 1) * BT]
                if eng_v:
                    nc.vector.tensor_add(dst, ptile[:], bcol.to_broadcast((P, BT)))
                else:
                    nc.scalar.add(dst, ptile[:], bcol)

            def make_osbs(s):
                return [
                    opool.tile([P, BC], mybir.dt.bfloat16, name=f"o{s}_{u}", tag="osb")
                    for u in range(4)
                ]

            # stage A
            klist, _ = STAGES[0]
            nk = len(klist)
            osbs = make_osbs(0)
            ptA = [
                [
                    (psa if u < 2 else psb).tile(
                        [P, BT], mybir.dt.float32, name=f"psA_{u}_{b}", tag="ps"
                    )
                    for b in range(NB)
                ]
                for u in range(4)
            ]
            for ko in range(nk):
                k = klist[ko]
                xt_t, kp = xmap[k]
                for u in range(4):
                    lhsT = mw[0][:, ko * 512 + u * P : ko * 512 + (u + 1) * P]
                    for b in range(NB):
                        nc.tensor.matmul(
                            ptA[u][b][:],
                            lhsT,
                            xt_t[:, kp, b * BT : (b + 1) * BT],
                            start=(ko == 0),
                            stop=(ko == nk - 1),
                        )
            # drain psa-half (u0,u1) first so stage B's first group can start
            for u in range(4):
                for b in range(NB):
                    drain(0, u, b, ptA[u][b], osbs, eng_v=(b == 0))
                nc.sync.dma_start(oT3[u], osbs[u][:])

            # stages B and C
            for s in (1, 2):
                klist, _ = STAGES[s]
                nk = len(klist)
                osbs = make_osbs(s)
                for b in range(NB):
                    pool = psa if b == 0 else psb
                    ptiles = [
                        pool.tile(
                            [P, BT], mybir.dt.float32, name=f"ps{s}_{b}_{u}", tag="ps"
                        )
                        for u in range(4)
                    ]
                    for ko in range(nk):
                        k = klist[ko]
                        xt_t, kp = xmap[k]
                        for u in range(4):
                            lhsT = mw[s][:, ko * 512 + u * P : ko * 512 + (u + 1) * P]
                            nc.tensor.matmul(
                                ptiles[u][:],
                                lhsT,
                                xt_t[:, kp, b * BT : (b + 1) * BT],
                                start=(ko == 0),
                                stop=(ko == nk - 1),
                            )
                    for u in range(4):
                        drain(s, u, b, ptiles[u], osbs, eng_v=(u % 2 == 0))
                        if b == NB - 1:
                            nc.sync.dma_start(oT3[s * 4 + u], osbs[u][:])

    nc.compile()
    return nc


def get_module():
    global _NC_CACHE
    if _NC_CACHE is None:
        _NC_CACHE = _build_module()
    return _NC_CACHE


def _pack_wm(arr2d, klist, ucol0):
    """arr2d (IN_DIM, UNITS)-like slab -> (128, len(klist)*512) packed
    [p, ko*512 + u] = arr2d[klist[ko]*128 + p, ucol0 + u]."""
    sl = arr2d[:, ucol0 : ucol0 + 512].reshape(K_CHUNKS, P, 512)[klist]
    return np.ascontiguousarray(sl.transpose(1, 0, 2).reshape(P, len(klist) * 512))


def make_in_maps(x, w, b, mask):
    x16 = x.astype(BF16)
    w16 = w.astype(BF16)
    m16T = np.ascontiguousarray(mask.astype(BF16).T)  # (3072, 1536)

    shared = {"warm": np.zeros((P, BT), dtype=BF16)}
    for s, (klist, ucol0) in enumerate(STAGES):
        shared[f"w{s}"] = _pack_wm(w16, klist, ucol0)
        shared[f"m{s}"] = _pack_wm(m16T, klist, ucol0)
    shared["bp"] = np.ascontiguousarray(b.astype(np.float32).reshape(12, P).T)

    in_maps = []
    for c in range(N_CORES):
        m = dict(shared)
        m["xT"] = np.ascontiguousarray(x16[c * BC : (c + 1) * BC].T)
        in_maps.append(m)
    return in_maps


def assemble(results):
    out = np.empty((BATCH, UNITS), dtype=np.float32)
    for c in range(N_CORES):
        out[c * BC : (c + 1) * BC, :] = results[c]["outT"].T
    return out


def kernel(x, w, b, mask, _trace=False, _trace_kwargs=None):
    x = np.asarray(x, dtype=np.float32)
    w = np.asarray(w, dtype=np.float32)
    b = np.asarray(b, dtype=np.float32)
    mask = np.asarray(mask, dtype=np.float32)
    nc = get_module()
    in_maps = make_in_maps(x, w, b, mask)
    res = run_bass_kernel_spmd(
        nc,
        in_maps,
        core_ids=list(range(N_CORES)),
        trace=_trace,
        **(_trace_kwargs or {}),
    )
    out = assemble(res.results)
    if _trace:
        return out, res
    return out
